# revision 4
# baseline (speedup 1.0000x reference)
"""ArcFace margin loss (ArcMarginLoss) on 8 Trainium2 NeuronCores.

Classification-parallel sharding: V=32000 classes split across 8 cores
(4000 each, padded to 4096).  The device kernel is a pure fp8 GEMM +
exp-rowsum pipeline; everything O(B*D) or O(V*D)-elementwise lives on the
host.

Host prep (numpy):
  - x-hat = x/|x|, w-hat = W/|W| rows (fp32), scaled by 16 and cast to
    fp8 e4m3.  PSUM then holds 256*cos, and the exp stage applies scale
    s/256 and bias -s, computing exp(s*cos - 30) directly: since cos <= 1
    no global max pass or cross-core collective is needed.
  - Both operands are packed K-major (contraction dim on partitions) for
    the PE's fp8 DoubleRow mode (two 128-deep k-planes per pass, 2x MAC
    throughput).  Weights are packed piece-major ([P, piece, j, i, 512])
    so each 512-class piece is one contiguous-per-partition DMA.
  - The label-column path (cos_y, phi = cos(theta+m), the per-row exp
    corrections) is O(B*D) and computed on the host in fp64.

Device per core (one SPMD NEFF), v2 pipeline:
  - Two phases of 2 class-quarters each; within a phase, m-tiles are
    walked with both quarter-chunks of one m adjacent.  Each chunk is a
    [128, 1024] fp32 PSUM tile filled by 4 DoubleRow matmuls (4-buf ring
    over all 8 PSUM banks).
  - Drains are split per m-tile between the two PSUM-capable engines
    (GpSimd has no PSUM port on TRN2):
      * scalar-m: ACT Exp with fused accum row-sum, in place (1 pass).
      * chain-m:  DVE tensor_scalar converts s*cos logits straight to
        int16 bf16 BIT CODES (Schraudolph exp in the bf16 domain) into
        SBUF scratch -- the single PSUM pass, so the PSUM slot frees
        after ~1.4us -- then one DVE tensor_reduce over the scratch read
        as bf16 (2-4x DVE rate for 16-bit) yields the pair's row-sum.
    The (ph, m) -> engine map is tuned so both engines stay under the PE
    fill rate; the first two m-tiles of phase 0 are walked q-major and
    drained as singles to hide the initial weight-DMA latency.
  - HAM warm-up: a few real DoubleRow matmuls on a zeroed fp8 tile ramp
    the PE clock gate (transposes don't count as PE-busy for HAM).
  - DMA prefix: weight pieces on the sync queue, x slabs interleaved, so
    the first matmul is gated only by piece 0 + the first x slab.
  - Output: per-chunk / per-pair row sums Spart [128, 4, 16] fp32.

Host epilogue: S = sum_c sum_q S_cq, scatter-add the label corrections,
loss = mean(30 + log(S) - s*phi_label).
"""

import math
import numpy as np
from contextlib import ExitStack

import concourse.bass as bass
import concourse.tile as tile
from concourse import bacc, mybir
from concourse import bass_utils
from concourse._compat import with_exitstack

P = 128
B = 2048          # batch rows
D = 512           # feature dim
V = 32000         # classes
NCORES = 8
VS = V // NCORES  # 4000 classes per core
VSP = 4096        # padded shard size
MT = B // P       # 16 batch row tiles
NJ = 2            # DoubleRow passes over D (each contracts 256)
NQ = 4            # class quarters per core
NPIECE = 8        # 512-class weight pieces per core
QW = (1024, 1024, 1024, 928)  # real columns per quarter (q3 trims the pad)

S_SCALE = 30.0
M_MARGIN = 0.5
SHIFT = 30.0      # exp(logit - SHIFT): logits <= 30 so always <= 0
WS = 16.0         # fp8 encode scale for x-hat and w-hat
EPS = 1e-12
ESC = S_SCALE / (WS * WS)   # psum -> logit scale (psum = 256*cos)

F32 = mybir.dt.float32
BF16 = mybir.dt.bfloat16
F8 = mybir.dt.float8e4
I16 = mybir.dt.int16
OP = mybir.AluOpType
AF = mybir.ActivationFunctionType
AX = mybir.AxisListType
DR = mybir.MatmulPerfMode.DoubleRow

# int16 Schraudolph: code = round(A16*psum + B16); code bits ARE the bf16
# representation of ~exp(ESC*psum - SHIFT).  The -7.3616 recenters the
# 2^frac-vs-exp sawtooth so the value-weighted mean ratio is 1 (calibrated
# numerically; residual is ~1.8% noise/elem that averages out per row).
_LOG2E = 1.4426950408889634
A16 = float(np.float32(ESC * _LOG2E * 128.0))
B16 = float(np.float32(128.0 * (127.0 - SHIFT * _LOG2E) - 7.3616))

# drain-mode map: for each phase, m-tile -> 'S' (scalar ACT-exp+accum) or
# 'C' (DVE int16-Schraudolph + bf16 reduce).  The first two m's of phase
# 0 are forced to q-major singles (see _arc_kernel) and use the modes
# below per-chunk.  Tuned so scalar+DVE both stay below the PE fill rate.
PREFIX_M = 2      # m-tiles of phase 0 walked q-major / drained as singles
MODE = [
    # phase 0: m0..m15
    ['C', 'C', 'S', 'C', 'S', 'C', 'S', 'C',
     'S', 'C', 'S', 'C', 'S', 'C', 'S', 'C'],
    # phase 1: m0..m15
    ['S', 'C', 'S', 'C', 'S', 'C', 'S', 'C',
     'S', 'C', 'S', 'C', 'S', 'C', 'S', 'S'],
]
N_WARM = 4        # HAM warm-up matmuls (N=512 DoubleRow each, ~0.5us)


@with_exitstack
def _arc_kernel(ctx: ExitStack, tc: tile.TileContext,
                xt_d: bass.AP, wt_d: bass.AP, s_d: bass.AP):
    nc = tc.nc

    sb = ctx.enter_context(tc.tile_pool(name="sb", bufs=1))
    scr_pool = ctx.enter_context(tc.tile_pool(name="scr", bufs=2))
    ps = ctx.enter_context(tc.tile_pool(name="ps", bufs=4, space="PSUM"))

    xT = sb.tile([P, MT, NJ, 2, P], F8)       # [p, m, j, i, c]
    wT = sb.tile([P, NPIECE, NJ, 2, 512], F8)  # [p, piece, j, i, v]
    Spart = sb.tile([P, NQ, MT], F32)          # per-chunk/pair row sums
    warm = sb.tile([P, 2, 512], F8)            # zeros for HAM warm-up MMs
    zt = sb.tile([P, 1], F32)
    nbias = sb.tile([P, 1], F32)               # -SHIFT bias for the exp

    nc.vector.memset(Spart, 0.0)
    nc.vector.memset(warm, 0.0)
    nc.vector.memset(zt, 0.0)
    nc.vector.memset(nbias, -SHIFT)

    # ---- DMA prefix -------------------------------------------------
    # sync queue: weight pieces with x slabs interleaved so piece 0 and
    # x[m0,m1] land first; everything else in need-order.
    nc.sync.dma_start(out=wT[:, 0, 0], in_=wt_d[:, 0, 0])   # p0 j0
    nc.sync.dma_start(out=wT[:, 0, 1], in_=wt_d[:, 0, 1])   # p0 j1
    nc.sync.dma_start(out=xT[:, 0:2], in_=xt_d[:, 0:2])     # x m0-1
    nc.sync.dma_start(out=wT[:, 1], in_=wt_d[:, 1])         # p1
    nc.sync.dma_start(out=wT[:, 2], in_=wt_d[:, 2])         # p2
    nc.sync.dma_start(out=wT[:, 3], in_=wt_d[:, 3])         # p3
    nc.sync.dma_start(out=xT[:, 2:6], in_=xt_d[:, 2:6])     # x m2-5
    nc.sync.dma_start(out=xT[:, 6:11], in_=xt_d[:, 6:11])   # x m6-10
    nc.sync.dma_start(out=xT[:, 11:16], in_=xt_d[:, 11:16])  # x m11-15
    for pc in range(4, NPIECE):
        nc.sync.dma_start(out=wT[:, pc], in_=wt_d[:, pc])

    # Force the Exp table load now (scalar queue is otherwise idle until
    # its first chunk ACT at ~13us; the load takes ~2.9us).
    e0 = sb.tile([P, 1], F32)
    nc.scalar.activation(out=e0, in_=zt, func=AF.Exp, bias=nbias)

    # HAM warm-up: real (zero-data) DoubleRow matmuls engage the PE
    # activity monitor while the first weight pieces land.
    wpm = ps.tile([P, 1024], F32, tag="mm", name="warm")
    for _ in range(N_WARM):
        nc.tensor.matmul(wpm[:, 0:512], warm[:, :, 0:128], warm,
                         start=True, stop=True, perf_mode=DR)

    # ---- chunk pipeline ---------------------------------------------
    def fill_chunk(m, q):
        """4 DoubleRow matmuls -> [128, w] fp32 psum chunk of (m, q)."""
        w = QW[q]
        pm = ps.tile([P, 1024], F32, tag="mm", name="pm")
        for j in range(NJ):
            for b in range(2):
                lo = b * 512
                hi = min(lo + 512, w)
                nc.tensor.matmul(
                    pm[:, lo:hi],
                    xT[:, m, j],
                    wT[:, 2 * q + b, j, :, 0:hi - lo],
                    start=(j == 0), stop=(j == NJ - 1),
                    perf_mode=DR)
        return pm, w

    def drain_scalar(pm, w, q, m):
        nc.scalar.activation(
            out=pm[:, :w], in_=pm[:, :w], func=AF.Exp,
            bias=nbias, scale=ESC,
            accum_out=Spart[:, q, m:m + 1])

    # phase 0 prefix: first PREFIX_M m-tiles q-major, drained per-chunk
    # (hides the p2/p3 weight-DMA latency behind 2*PREFIX_M chunks).
    for qq in range(2):
        for m in range(PREFIX_M):
            pm, w = fill_chunk(m, qq)
            if MODE[0][m] == 'S':
                drain_scalar(pm, w, qq, m)
            else:
                scr = scr_pool.tile([P, 2048], I16, name="scr")
                nc.vector.tensor_scalar(scr[:, 0:w], pm[:, :w],
                                        A16, B16, OP.mult, OP.add)
                nc.vector.tensor_reduce(
                    out=Spart[:, qq, m:m + 1], in_=scr[:, 0:w].bitcast(BF16),
                    axis=AX.X, op=OP.add)

    # main loop: both quarter-chunks of an m-tile adjacent.
    for ph in range(2):
        ms = range(PREFIX_M, MT) if ph == 0 else range(MT)
        for m in ms:
            mode = MODE[ph][m]
            if mode == 'C':
                scr = scr_pool.tile([P, 2048], I16, name="scr")
            wtot = 0
            for qq in range(2):
                q = 2 * ph + qq
                pm, w = fill_chunk(m, q)
                if mode == 'S':
                    drain_scalar(pm, w, q, m)
                else:
                    nc.vector.tensor_scalar(scr[:, wtot:wtot + w], pm[:, :w],
                                            A16, B16, OP.mult, OP.add)
                    wtot += w
            if mode == 'C':
                nc.vector.tensor_reduce(
                    out=Spart[:, 2 * ph, m:m + 1],
                    in_=scr[:, 0:wtot].bitcast(BF16),
                    axis=AX.X, op=OP.add)

    nc.sync.dma_start(out=s_d.rearrange("(p q m) -> p q m", p=P, q=NQ),
                      in_=Spart)


def build_bass():
    nc = bacc.Bacc("TRN2", target_bir_lowering=False, debug=False,
                   enable_asserts=False, num_devices=NCORES)
    xt_d = nc.dram_tensor("xt_in", [P, MT, NJ, 2, P], F8,
                          kind="ExternalInput").ap()
    wt_d = nc.dram_tensor("wt_in", [P, NPIECE, NJ, 2, 512], F8,
                          kind="ExternalInput").ap()
    s_d = nc.dram_tensor("s_out", [NQ * B], F32, kind="ExternalOutput").ap()
    with tile.TileContext(nc) as tc:
        _arc_kernel(tc, xt_d, wt_d, s_d)
    nc.compile()
    return nc


_NC = None


def _get_nc():
    global _NC
    if _NC is None:
        _NC = build_bass()
    return _NC


def make_in_maps(xn: np.ndarray, W: np.ndarray):
    import ml_dtypes
    F8NP = ml_dtypes.float8_e4m3

    xq = (xn * WS).astype(F8NP)                      # [B, D]
    # xt[p, m, j, i, c] = xq[m*128 + c, j*256 + i*128 + p]
    xt = np.ascontiguousarray(
        xq.reshape(MT, P, NJ, 2, P).transpose(4, 0, 2, 3, 1))

    wnorm = np.linalg.norm(W, axis=1, keepdims=True)
    Wn = W / np.maximum(wnorm, EPS)
    in_maps = []
    for c in range(NCORES):
        wq = np.zeros((VSP, D), dtype=F8NP)
        wq[:VS] = (Wn[c * VS:(c + 1) * VS] * WS).astype(F8NP)
        # wt[p, piece, j, i, v] = wq[piece*512 + v, j*256 + i*128 + p]
        wt = np.ascontiguousarray(
            wq.reshape(NPIECE, 512, NJ, 2, P).transpose(4, 0, 2, 3, 1))
        in_maps.append({"xt_in": xt, "wt_in": wt})
    return in_maps, Wn


def kernel(x, W, labels, **run_kwargs):
    x = np.ascontiguousarray(np.asarray(x), dtype=np.float32)
    W = np.ascontiguousarray(np.asarray(W), dtype=np.float32)
    lab = np.asarray(labels).astype(np.int64)
    assert x.shape == (B, D) and W.shape == (V, D) and lab.shape == (B,), \
        (x.shape, W.shape, lab.shape)

    xn = x / np.maximum(np.linalg.norm(x, axis=1, keepdims=True), EPS)

    nc = _get_nc()
    in_maps, Wn = make_in_maps(xn, W)
    res = bass_utils.run_bass_kernel_spmd(
        nc, in_maps, core_ids=list(range(NCORES)), **run_kwargs)

    S = np.zeros(B, dtype=np.float64)
    for r in res.results:
        sp = r["s_out"].reshape(P, NQ, MT).sum(axis=1)  # add the quarters
        S += sp.T.reshape(-1).astype(np.float64)

    # Host label-column correction (O(B*D), fp64 epilogue).
    cos_y = np.einsum("bd,bd->b", xn.astype(np.float64),
                      Wn[lab].astype(np.float64))
    sin_y = np.sqrt(np.clip(1.0 - cos_y * cos_y, 0.0, 1.0))
    phi_y = cos_y * math.cos(M_MARGIN) - sin_y * math.sin(M_MARGIN)
    S += np.exp(S_SCALE * phi_y - SHIFT) - np.exp(S_SCALE * cos_y - SHIFT)
    loss = np.mean(SHIFT + np.log(S) - S_SCALE * phi_y)

    kernel.last_results = res
    return np.asarray(loss, dtype=np.float32)


# revision 9
# speedup vs baseline: 1.3644x; 1.3644x over previous
"""ArcFace margin loss (ArcMarginLoss) on 8 Trainium2 NeuronCores.

Classification-parallel sharding: V=32000 classes split across 8 cores
(4000 each, padded to 4096).  The device kernel is a pure fp8 GEMM +
exp-rowsum pipeline; everything O(B*D) or O(V*D)-elementwise lives on the
host.

Host prep (numpy):
  - x-hat = x/|x|, w-hat = W/|W| rows (fp32), scaled by 16 and cast to
    fp8 e4m3.  PSUM then holds 256*cos, and the exp stage applies scale
    s/256 and bias -s, computing exp(s*cos - 30) directly: since cos <= 1
    no global max pass or cross-core collective is needed.
  - Both operands are packed K-major for the PE's fp8 DoubleRow mode
    (2x MAC throughput).  Weights are packed piece-major
    ([P, piece, j, i, 512]) so each 512-class piece is one
    contiguous-per-partition DMA.
  - The label-column path (cos_y, phi, per-row exp corrections) is
    O(B*D), computed on the host in fp64.

Device per core (one SPMD NEFF), v3 pipeline:
  - Two phases of 2 class-quarters each.  Per (phase, m-tile): a single
    [128, 2048] fp32 PSUM tile (4 banks; 2-buf ring = whole PSUM) filled
    by 8 DoubleRow matmuls (4 x 512-col pieces x 2 k-planes).
  - Drains are split per m-tile between the two PSUM-capable engines
    (GpSimd has no PSUM port on TRN2):
      * 'S': ONE wide scalar ACT Exp over all 2048 cols with fused accum
        row-sum (amortizes the 352-cycle ACT ramp + 344ns accumulator
        read); in-place, and the 2-buf ring leaves 2072ns of drain
        latency slack so the 2344ns ACT fits without stalling the PE.
      * 'C': DVE tensor_scalar converts each 1024-col half of the s*cos
        logits straight to int16 bf16 BIT CODES (Schraudolph exp in the
        bf16 domain) into SBUF scratch -- the PSUM halves free after
        ~1.6us each -- then one fused tensor_scalar(mult 1, add 0) with
        accum_out over the codes read as bf16 yields the row-sum without
        a separate reduce pass.
    The (ph, m) -> engine map is tuned so both engines stay below the PE
    fill rate (~2072ns per m).
  - The first two m-tiles of phase 0 are walked q-major to hide the
    initial weight-DMA latency; DMA issues are split across the sync and
    scalar hardware-DGE queues so the critical pieces transfer in
    parallel.
  - HAM warm-up: real DoubleRow matmuls on a zeroed fp8 tile ramp the PE
    clock gate (transposes don't count as PE-busy for HAM).
  - Output: per-(phase, m) row sums Spart [128, 4, 16] fp32, DMA'd per
    phase.

Host epilogue: S = sum_c sum_q S_cq, scatter-add the label corrections,
loss = mean(30 + log(S) - s*phi_label).
"""

import math
import numpy as np
from contextlib import ExitStack

import concourse.bass as bass
import concourse.tile as tile
from concourse import bacc, mybir
from concourse import bass_utils
from concourse._compat import with_exitstack

P = 128
B = 2048          # batch rows
D = 512           # feature dim
V = 32000         # classes
NCORES = 8
VS = V // NCORES  # 4000 classes per core
VSP = 4096        # padded shard size
MT = B // P       # 16 batch row tiles
NJ = 2            # DoubleRow passes over D (each contracts 256)
NQ = 4            # class quarters per core
NPIECE = 8        # 512-class weight pieces per core
PW = (512, 512, 512, 512, 512, 512, 512, 416)  # real cols per piece

S_SCALE = 30.0
M_MARGIN = 0.5
SHIFT = 30.0      # exp(logit - SHIFT): logits <= 30 so always <= 0
WS = 16.0         # fp8 encode scale for x-hat and w-hat
EPS = 1e-12
ESC = S_SCALE / (WS * WS)   # psum -> logit scale (psum = 256*cos)

F32 = mybir.dt.float32
BF16 = mybir.dt.bfloat16
F8 = mybir.dt.float8e4
I16 = mybir.dt.int16
OP = mybir.AluOpType
AF = mybir.ActivationFunctionType
AX = mybir.AxisListType
DR = mybir.MatmulPerfMode.DoubleRow

# int16 Schraudolph: code = round(A16*psum + B16); code bits ARE the bf16
# representation of ~exp(ESC*psum - SHIFT).  The -7.3616 recenters the
# 2^frac-vs-exp sawtooth so the value-weighted mean ratio is 1 (calibrated
# numerically; residual is ~1.8% noise/elem that averages out per row).
_LOG2E = 1.4426950408889634
A16 = float(np.float32(ESC * _LOG2E * 128.0))
B16 = float(np.float32(128.0 * (127.0 - SHIFT * _LOG2E) - 7.3616))

# drain-mode map per (phase, m): 'S' = wide scalar ACT-exp+accum,
# 'C' = DVE wide int16-Schraudolph TS + DVE fused accum pass,
# 'D' = DVE wide TS + scalar ACT-identity accum over the codes.
# Measured: S = 2.33+0.34us scalar; C = 1.78us + 2.09+0.10us DVE;
# D = 1.78us DVE + 2.00+0.34us scalar.  Tuned so both engines stay
# under the PE fill rate of ~2.07us per m-tile.
MODE = [
    ['S', 'C', 'S', 'C', 'S', 'D', 'S', 'C',
     'S', 'S', 'C', 'S', 'C', 'S', 'C', 'S'],
    ['S', 'C', 'S', 'C', 'S', 'C', 'S', 'S',
     'C', 'S', 'S', 'C', 'S', 'C', 'S', 'S'],
]
PREFIX_M = 2      # m-tiles of phase 0 walked q-major (DMA latency hiding)
N_WARM = 7        # HAM warm-up matmuls (N=512 DoubleRow each, ~0.5us)
TS2_FUSED = True  # use tensor_scalar+accum_out for the chain row-sum


@with_exitstack
def _arc_kernel(ctx: ExitStack, tc: tile.TileContext,
                xt_d: bass.AP, wt_d: bass.AP, s_d: bass.AP):
    nc = tc.nc

    sb = ctx.enter_context(tc.tile_pool(name="sb", bufs=1))
    scr_pool = ctx.enter_context(tc.tile_pool(name="scr", bufs=2))
    ps = ctx.enter_context(tc.tile_pool(name="ps", bufs=2, space="PSUM"))

    xT = sb.tile([P, MT, NJ, 2, P], F8)        # [p, m, j, i, c]
    wT = sb.tile([P, NPIECE, NJ, 2, 512], F8)  # [p, piece, j, i, v]
    Spart = sb.tile([P, NQ, MT], F32)          # per-(ph, m) row sums
    warm = sb.tile([P, 2, 512], F8)            # zeros for HAM warm-up MMs
    zt = sb.tile([P, 1], F32)
    nbias = sb.tile([P, 1], F32)               # -SHIFT bias for the exp

    nc.gpsimd.memset(warm, 0.0)
    nc.vector.memset(Spart, 0.0)
    nc.vector.memset(zt, 0.0)
    nc.vector.memset(nbias, -SHIFT)

    # ---- DMA prefix -------------------------------------------------
    # Two hardware-DGE queues in parallel: sync carries most weight
    # pieces, scalar carries the first x slab + piece 2, so the first
    # m-tiles' operands land with minimum serialization.
    nc.sync.dma_start(out=wT[:, 0, 0], in_=wt_d[:, 0, 0])    # p0 j0
    nc.sync.dma_start(out=wT[:, 0, 1], in_=wt_d[:, 0, 1])    # p0 j1
    nc.sync.dma_start(out=wT[:, 1], in_=wt_d[:, 1])          # p1
    nc.sync.dma_start(out=wT[:, 3], in_=wt_d[:, 3])          # p3
    nc.sync.dma_start(out=xT[:, 2:6], in_=xt_d[:, 2:6])      # x m2-5
    nc.sync.dma_start(out=wT[:, 4], in_=wt_d[:, 4])          # p4
    nc.sync.dma_start(out=wT[:, 5], in_=wt_d[:, 5])          # p5
    nc.sync.dma_start(out=xT[:, 6:11], in_=xt_d[:, 6:11])    # x m6-10
    nc.sync.dma_start(out=wT[:, 6], in_=wt_d[:, 6])          # p6
    nc.sync.dma_start(out=wT[:, 7], in_=wt_d[:, 7])          # p7
    nc.sync.dma_start(out=xT[:, 11:16], in_=xt_d[:, 11:16])  # x m11-15

    nc.scalar.dma_start(out=xT[:, 0:2], in_=xt_d[:, 0:2])    # x m0-1
    nc.scalar.dma_start(out=wT[:, 2], in_=wt_d[:, 2])        # p2

    # Force the Exp table load now (~2.9us) so it's resident before the
    # first chunk ACT at ~13us.
    e0 = sb.tile([P, 1], F32)
    nc.scalar.activation(out=e0, in_=zt, func=AF.Exp, bias=nbias)

    # HAM warm-up: real (zero-data) DoubleRow matmuls engage the PE
    # activity monitor while the first weight pieces land.
    wpm = ps.tile([P, 2048], F32, tag="mm", name="warm")
    for _ in range(N_WARM):
        nc.tensor.matmul(wpm[:, 0:512], warm[:, :, 0:128], warm,
                         start=True, stop=True, perf_mode=DR)

    # ---- m-tile pipeline --------------------------------------------
    def fill_span(pm, ph, m, s_list):
        """DoubleRow matmuls for 512-col pieces s_list; j outer so 2-4
        consecutive matmuls share the same stationary x tile."""
        for j in range(NJ):
            for s in s_list:
                pc = 4 * ph + s
                w = PW[pc]
                nc.tensor.matmul(
                    pm[:, 512 * s:512 * s + w],
                    xT[:, m, j],
                    wT[:, pc, j, :, 0:w],
                    start=(j == 0), stop=(j == NJ - 1),
                    perf_mode=DR)

    # dummy SBUF sink for the scalar exp output (we only need accum_out;
    # writing PSUM in-place costs ~300ns of same-bank RW conflict).
    esink = sb.tile([P, 2048], BF16)

    def drain_m(ph, m, pm, scr, wtot):
        """Row-sum the filled [128, wtot] psum tile of (ph, m)."""
        mode = MODE[ph][m]
        if mode == 'S':
            nc.scalar.activation(
                out=esink[:, :wtot], in_=pm[:, :wtot], func=AF.Exp,
                bias=nbias, scale=ESC,
                accum_out=Spart[:, 2 * ph, m:m + 1])
            return
        nc.vector.tensor_scalar(scr[:, 0:wtot], pm[:, 0:wtot],
                                A16, B16, OP.mult, OP.add)
        codes = scr[:, 0:wtot].bitcast(BF16)
        if mode == 'D':
            nc.scalar.activation(
                out=codes, in_=codes, func=AF.Identity, bias=zt,
                accum_out=Spart[:, 2 * ph, m:m + 1])
        elif TS2_FUSED:
            nc.vector.tensor_scalar(
                codes, codes, 1.0, 0.0, OP.mult, OP.add,
                accum_out=Spart[:, 2 * ph, m:m + 1])
        else:
            nc.vector.tensor_reduce(
                out=Spart[:, 2 * ph, m:m + 1], in_=codes,
                axis=AX.X, op=OP.add)

    s_view = s_d.rearrange("(p q m) -> p q m", p=P, q=NQ)

    for ph in range(2):
        wtot = 2048 if ph == 0 else 1952
        if ph == 0:
            # prefix: m0/m1 interleaved q-major to hide the weight DMA;
            # per-half (narrow) drains so the psum bufs free in time.
            pms = [ps.tile([P, 2048], F32, tag="mm", name=f"pm{m}")
                   for m in range(PREFIX_M)]
            scrs = [scr_pool.tile([P, 2048], I16, name=f"scr{m}")
                    if MODE[0][m] != 'S' else None for m in range(PREFIX_M)]
            for qq in range(2):
                for m in range(PREFIX_M):
                    fill_span(pms[m], 0, m, (2 * qq, 2 * qq + 1))
                    half = slice(1024 * qq, 1024 * qq + 1024)
                    if MODE[0][m] == 'S':
                        nc.scalar.activation(
                            out=esink[:, 0:1024], in_=pms[m][:, half],
                            func=AF.Exp, bias=nbias, scale=ESC,
                            accum_out=Spart[:, qq, m:m + 1])
                    else:
                        nc.vector.tensor_scalar(
                            scrs[m][:, half], pms[m][:, half],
                            A16, B16, OP.mult, OP.add)
            for m in range(PREFIX_M):
                if MODE[0][m] == 'S':
                    continue
                codes = scrs[m][:, 0:wtot].bitcast(BF16)
                if TS2_FUSED:
                    nc.vector.tensor_scalar(
                        codes, codes, 1.0, 0.0, OP.mult, OP.add,
                        accum_out=Spart[:, 0, m:m + 1])
                else:
                    nc.vector.tensor_reduce(
                        out=Spart[:, 0, m:m + 1], in_=codes,
                        axis=AX.X, op=OP.add)
            ms = range(PREFIX_M, MT)
        else:
            ms = range(MT)
        for m in ms:
            pm = ps.tile([P, 2048], F32, tag="mm", name="pm")
            scr = (scr_pool.tile([P, 2048], I16, name="scr")
                   if MODE[ph][m] != 'S' else None)
            fill_span(pm, ph, m, (0, 1, 2, 3))
            drain_m(ph, m, pm, scr, wtot)
        # ship this phase's sums while the next phase runs
        nc.sync.dma_start(out=s_view[:, 2 * ph:2 * ph + 2],
                          in_=Spart[:, 2 * ph:2 * ph + 2])


def build_bass():
    nc = bacc.Bacc("TRN2", target_bir_lowering=False, debug=False,
                   enable_asserts=False, num_devices=NCORES)
    xt_d = nc.dram_tensor("xt_in", [P, MT, NJ, 2, P], F8,
                          kind="ExternalInput").ap()
    wt_d = nc.dram_tensor("wt_in", [P, NPIECE, NJ, 2, 512], F8,
                          kind="ExternalInput").ap()
    s_d = nc.dram_tensor("s_out", [NQ * B], F32, kind="ExternalOutput").ap()
    with tile.TileContext(nc) as tc:
        _arc_kernel(tc, xt_d, wt_d, s_d)
    nc.compile()
    return nc


_NC = None


def _get_nc():
    global _NC
    if _NC is None:
        _NC = build_bass()
    return _NC


def make_in_maps(xn: np.ndarray, W: np.ndarray):
    import ml_dtypes
    F8NP = ml_dtypes.float8_e4m3

    xq = (xn * WS).astype(F8NP)                      # [B, D]
    # xt[p, m, j, i, c] = xq[m*128 + c, j*256 + i*128 + p]
    xt = np.ascontiguousarray(
        xq.reshape(MT, P, NJ, 2, P).transpose(4, 0, 2, 3, 1))

    wnorm = np.linalg.norm(W, axis=1, keepdims=True)
    Wn = W / np.maximum(wnorm, EPS)
    in_maps = []
    for c in range(NCORES):
        wq = np.zeros((VSP, D), dtype=F8NP)
        wq[:VS] = (Wn[c * VS:(c + 1) * VS] * WS).astype(F8NP)
        # wt[p, piece, j, i, v] = wq[piece*512 + v, j*256 + i*128 + p]
        wt = np.ascontiguousarray(
            wq.reshape(NPIECE, 512, NJ, 2, P).transpose(4, 0, 2, 3, 1))
        in_maps.append({"xt_in": xt, "wt_in": wt})
    return in_maps, Wn


def kernel(x, W, labels, **run_kwargs):
    x = np.ascontiguousarray(np.asarray(x), dtype=np.float32)
    W = np.ascontiguousarray(np.asarray(W), dtype=np.float32)
    lab = np.asarray(labels).astype(np.int64)
    assert x.shape == (B, D) and W.shape == (V, D) and lab.shape == (B,), \
        (x.shape, W.shape, lab.shape)

    xn = x / np.maximum(np.linalg.norm(x, axis=1, keepdims=True), EPS)

    nc = _get_nc()
    in_maps, Wn = make_in_maps(xn, W)
    res = bass_utils.run_bass_kernel_spmd(
        nc, in_maps, core_ids=list(range(NCORES)), **run_kwargs)

    S = np.zeros(B, dtype=np.float64)
    for r in res.results:
        sp = r["s_out"].reshape(P, NQ, MT).sum(axis=1)  # add the quarters
        S += sp.T.reshape(-1).astype(np.float64)

    # Host label-column correction (O(B*D), fp64 epilogue).
    cos_y = np.einsum("bd,bd->b", xn.astype(np.float64),
                      Wn[lab].astype(np.float64))
    sin_y = np.sqrt(np.clip(1.0 - cos_y * cos_y, 0.0, 1.0))
    phi_y = cos_y * math.cos(M_MARGIN) - sin_y * math.sin(M_MARGIN)
    S += np.exp(S_SCALE * phi_y - SHIFT) - np.exp(S_SCALE * cos_y - SHIFT)
    loss = np.mean(SHIFT + np.log(S) - S_SCALE * phi_y)

    kernel.last_results = res
    return np.asarray(loss, dtype=np.float32)
